# revision 35
# baseline (speedup 1.0000x reference)
"""Multi-head attention with RoPE on 8 Trainium2 NeuronCores.

Problem: B=4, L=2048, D=1024, H=16 heads of dim 64, fp32, full (non-causal)
softmax attention with concatenated-halves RoPE on q and k.

Sharding: tensor-parallel over heads. Each of the 8 cores owns 2 heads:
 - computes q/k/v projections for its heads only (W_qkv column slice),
 - runs attention for its 2 heads x 4 batches,
 - computes a rank-128 partial of the output projection (W_proj row slice).
The host sums the 8 partial outputs (the only cross-core reduction).

On-core layout choices (v2 — engine-rebalanced):
 - q, k are produced FEATURE-major ([head_dim, tokens]) directly by the QKV
   GEMM (weights pre-transposed/permuted on host), so the QK^T matmul needs
   no transposes. RoPE's even/odd feature split is pre-applied as a row
   permutation of W_q/W_k; RoPE = 2 DVE muls + a 32-partition-block DMA swap
   + a Pool (gpsimd) add, keeping the ACT engine free.
 - scores are computed TRANSPOSED ([k_tokens, q_tokens]); softmax exp is the
   ONLY work on ACT (scale folded into W_q on the host). The denominator
   comes free as a ones-column in the p@v stationary, placed FIRST (psum
   partition 0) so reciprocal_approx_fast can read it straight from PSUM.
 - v is produced feature-major then moved token-major by DMA XBAR transposes
   (no PE transposes, no PSUM traffic). v tiles are padded to 80 cols/kj so
   every transpose lands 16-column-aligned; stationary reads cols 15:80
   (ones at 15, v at 16:80).
 - softmax normalization is fused into the PSUM->SBUF copy of the attention
   output: ao = o_psum * broadcast(1/denom) in one DVE tensor_mul per head.
 - matmul operands are bf16 (PE streams 1 cycle/row); accumulation fp32 in
   PSUM; softmax/rope/normalization math fp32.
 - input/weight/output DMAs are split fine-grained and alternated across the
   two HWDGE queues (sync + scalar) so the opening GEMM starts ~1us in and
   no queue sees head-of-line blocking.
 - emission is software-pipelined: phase1 of batch b+1 and the output
   projection of batch b are emitted between the attention chunks of batch
   b so the Tile scheduler interleaves them into ACT-bound gaps.
"""

import sys

for _p in ("/opt/trn_rl_repo",):
    if _p not in sys.path:
        sys.path.insert(0, _p)

import numpy as np
import concourse.bass as bass
import concourse.mybir as mybir
from concourse import bacc
from concourse.tile import TileContext
from concourse.bass_utils import run_bass_kernel_spmd
from concourse.masks import make_identity

F32 = mybir.dt.float32
import ml_dtypes
F16 = mybir.dt.bfloat16
RF16 = mybir.dt.float16

B, L, D = 4, 2048, 1024
H, HD = 16, 64
NCORES = 8
HPC = H // NCORES  # 2 heads per core
TOK = B * L
BLK = 512  # gemm moving-dim block
QBLK = 512  # attention query block (one PSUM bank of fp32 output)
NBLK = L // BLK  # 4
NQB = L // QBLK  # 4
KT = D // 128  # 8 contraction tiles for the qkv projection
NKJ = L // 128  # 16 key tiles per batch
VW = 65  # v-tile width per kj block: v at 0:64, ones column at 64
ROPE_BASE = 10000.0

Exp = mybir.ActivationFunctionType.Exp


class _Ctx:
    pass


def _build_program():
    nc = bacc.Bacc("TRN2", target_bir_lowering=False, debug=False)

    c = _Ctx()
    c.nc = nc
    c.xt_d = nc.dram_tensor("xt", [D, TOK], F16, kind="ExternalInput")
    c.wqk_d = nc.dram_tensor("wqk", [D, 256], F16, kind="ExternalInput")
    c.wv_d = nc.dram_tensor("wv", [D, 128], F16, kind="ExternalInput")
    c.wp_d = nc.dram_tensor("wp", [128, D], F16, kind="ExternalInput")
    c.cc_d = nc.dram_tensor("cc", [128, L], F32, kind="ExternalInput")
    c.ssw_d = nc.dram_tensor("ssw", [128, L], F32, kind="ExternalInput")
    c.out_d = nc.dram_tensor("out", [B, D, L], RF16, kind="ExternalOutput")

    with TileContext(nc) as tc:
        with (
            tc.tile_pool(name="singles", bufs=1) as singles,
            tc.tile_pool(name="xin", bufs=2) as xin,
            tc.tile_pool(name="batch", bufs=2) as batch,
            tc.tile_pool(name="rope", bufs=4) as rope,
            tc.tile_pool(name="pexp", bufs=6) as pexp,
            tc.tile_pool(name="norm", bufs=4) as norm,
            tc.tile_pool(name="outp", bufs=6) as outp,
            tc.tile_pool(name="ps_g", bufs=2, space="PSUM") as ps_g,
            tc.tile_pool(name="ps_s", bufs=2, space="PSUM") as ps_s,
            tc.tile_pool(name="ps_o", bufs=2, space="PSUM") as ps_o,
        ):
            c.xin, c.batch, c.rope = xin, batch, rope
            c.pexp, c.norm, c.outp = pexp, norm, outp
            c.ps_g, c.ps_s, c.ps_o = ps_g, ps_s, ps_o

            # Resident weights / tables. Queue plan:
            #   sync:   wqk (per-kd), wv, cc/ssw (per-blk interleaved), wp
            #   scalar: x batch 0 (first block per-kd)
            # so the opening q-gemm starts as soon as wqk[kd0] + x[kd0] land.
            c.wqk_sb = singles.tile([128, KT, 256], F16, tag="wqk")
            wqk_r = c.wqk_d[:, :].rearrange("(k p) e -> p k e", p=128)
            for kd in range(KT):
                nc.sync.dma_start(out=c.wqk_sb[:, kd, :], in_=wqk_r[:, kd, :])
            c.bt = {}
            _issue_x(c, 0)
            c.wv_sb = singles.tile([128, KT, 128], F16, tag="wv")
            nc.sync.dma_start(
                out=c.wv_sb[:], in_=c.wv_d[:, :].rearrange("(k p) e -> p k e", p=128)
            )
            c.cc_sb = singles.tile([128, L], F32, tag="cc")
            c.ssw_sb = singles.tile([128, L], F32, tag="ssw")

            def _issue_tables(blk):
                ts = slice(blk * BLK, (blk + 1) * BLK)
                nc.sync.dma_start(out=c.cc_sb[:, ts], in_=c.cc_d[:, ts])
                nc.sync.dma_start(out=c.ssw_sb[:, ts], in_=c.ssw_d[:, ts])
                if blk > 0:
                    t0 = c.bt[0]
                    x_r = c.xt_d[:, 0:L].rearrange("(k p) t -> p k t", p=128)
                    nc.sync.dma_start(out=t0.x_t[:, :, ts], in_=x_r[:, :, ts])

            c.wp_sb = singles.tile([128, D], F16, tag="wp")
            c.ident = singles.tile([128, 128], F16, tag="ident")
            make_identity(nc, c.ident[:])

            # Software-pipelined emission. Batch 0's phase1 runs standalone
            # (q first on block 0 so attention can start; k before q on later
            # blocks since scores consume every k block in qi order); the v
            # transposes ride right behind each v_fm block. Rope tables land
            # just-in-time so the sync queue reaches the swap DMAs without
            # backlog.
            _vinit(c, 0)
            for blk in range(NBLK):
                _issue_tables(blk)
                order = "qkv" if blk == 0 else "kvq"
                for g in order:
                    _gemm_group(c, 0, blk, g)
                    if g == "v":
                        _vtrans_part(c, 0, blk)
            nc.sync.dma_start(out=c.wp_sb[:], in_=c.wp_d[:, :])

            # Main loop: each phase2 kj-stream carries interleaved "filler"
            # PE work (next batch's gemm groups, v transposes, and the
            # previous chunk's projection) so the Tensor engine has
            # exp-independent matmuls to run while ACT catches up.
            for b in range(B):
                for qi in range(NQB):
                    fillers = {}
                    if b + 1 < B:
                        if qi == 0:
                            _vinit(c, b + 1)
                            _issue_x(c, b + 1)

                        def mk(g, bb=b + 1, blkx=qi):
                            def f():
                                _gemm_group(c, bb, blkx, g)
                                if g == "v":
                                    _vtrans_part(c, bb, blkx)
                            return f

                        fillers[3] = [mk("v")]
                        fillers[6] = [mk("q")]
                        fillers[9] = [mk("k")]
                    if qi >= 1:
                        pb, pblk = b, qi - 1
                    elif b > 0:
                        pb, pblk = b - 1, NQB - 1
                    else:
                        pb = None
                    if pb is not None:
                        fillers.setdefault(12, []).append(
                            lambda bb=pb, blkx=pblk: _phase3_part(c, bb, blkx, 0, 4)
                        )
                        fillers.setdefault(15, []).append(
                            lambda bb=pb, blkx=pblk: _phase3_part(c, bb, blkx, 4, 8)
                        )
                    _phase2_chunk(c, b, qi, fillers)
            _phase3_chunk(c, B - 1, NQB - 1)

    nc.compile()
    return nc


def _tiles(c, b):
    if b not in c.bt:
        t = _Ctx()
        t.q_ro = c.batch.tile([128, L], F16, tag="qro")
        t.k_ro = c.batch.tile([128, L], F16, tag="kro")
        t.v_fm = c.batch.tile([128, L], F16, tag="vfm")
        t.v0 = c.batch.tile([128, NKJ, VW], F16, tag="v0")
        t.v1 = c.batch.tile([128, NKJ, VW], F16, tag="v1")
        t.ao = c.batch.tile([128, L], F16, tag="ao")
        t.x_t = None
        c.bt[b] = t
    return c.bt[b]


def _issue_x(c, b):
    # chunked x load on the scalar queue (sync carries weights/tables/etc).
    # Batch 0's first block is issued per-kd so the opening gemm only waits
    # for ~128KB; later chunks keep full prefetch lead time.
    nc = c.nc
    t = _tiles(c, b)
    t.x_t = c.xin.tile([128, KT, L], F16, tag="x")
    x_r = c.xt_d[:, b * L : (b + 1) * L].rearrange("(k p) t -> p k t", p=128)
    if b == 0:
        # fine pieces for the opening gemm; blocks 1-3 are emitted by the
        # phase1 loop (on sync, interleaved with the rope tables) so the
        # scalar queue reaches batch 0's swap DMAs early
        for kd in range(KT):
            nc.scalar.dma_start(
                out=t.x_t[:, kd, 0:BLK], in_=x_r[:, kd, 0:BLK]
            )
        return
    for ck in range(NBLK):
        nc.scalar.dma_start(
            out=t.x_t[:, :, ck * BLK : (ck + 1) * BLK],
            in_=x_r[:, :, ck * BLK : (ck + 1) * BLK],
        )


def _gemm_group(c, b, blk, g):
    nc = c.nc
    t = _tiles(c, b)
    ts = slice(blk * BLK, (blk + 1) * BLK)
    if t.x_t is None:
        _issue_x(c, b)
    if g in ("q", "k"):
        wcol, dst = (0, t.q_ro) if g == "q" else (128, t.k_ro)
        ps = c.ps_g.tile([128, BLK], F32, tag="g")
        for kd in range(KT):
            nc.tensor.matmul(
                ps[:],
                c.wqk_sb[:, kd, wcol : wcol + 128],
                t.x_t[:, kd, ts],
                start=(kd == 0),
                stop=(kd == KT - 1),
            )
        # rope: dst = ps*cc + blockswap(ps*ssw); muls+add on DVE, swap via
        # SBUF->SBUF DMA on the sync queue.
        tmp_c = c.rope.tile([128, BLK], F32, tag="tc")
        nc.vector.tensor_mul(tmp_c[:], ps[:], c.cc_sb[:, ts])
        tmp_s = c.rope.tile([128, BLK], F32, tag="tsn")
        nc.vector.tensor_mul(tmp_s[:], ps[:], c.ssw_sb[:, ts])
        tmp_w = c.rope.tile([128, BLK], F32, tag="tw")
        # swaps ride the scalar queue: x-prefetch traffic there is dep-free,
        # so swap DMAs never queue behind out-DMAs whose producer casts may
        # lag (FIFO head-of-line convoys).
        for a, bb in ((0, 32), (32, 0), (64, 96), (96, 64)):
            nc.scalar.dma_start(out=tmp_w[a : a + 32, :], in_=tmp_s[bb : bb + 32, :])
        nc.vector.tensor_add(dst[:, ts], tmp_c[:], tmp_w[:])
    else:
        psv = c.ps_g.tile([128, BLK], F32, tag="g")
        for kd in range(KT):
            nc.tensor.matmul(
                psv[:],
                c.wv_sb[:, kd, :],
                t.x_t[:, kd, ts],
                start=(kd == 0),
                stop=(kd == KT - 1),
            )
        # during batch 0's phase1 ACT is idle (no exp yet) — use it so the
        # DVE backlog doesn't delay k_ro/v readiness for the first phase2
        if b == 0:
            nc.scalar.copy(t.v_fm[:, ts], psv[:])
        else:
            nc.vector.tensor_copy(t.v_fm[:, ts], psv[:])


def _vinit(c, b):
    nc = c.nc
    t = _tiles(c, b)
    nc.gpsimd.memset(t.v0[:, :, 64], 1.0)
    nc.gpsimd.memset(t.v1[:, :, 64], 1.0)


def _vtrans_part(c, b, blk):
    nc = c.nc
    t = _tiles(c, b)
    for tt in range(blk * 4, blk * 4 + 4):
        pst = c.ps_g.tile([128, 128], F16, tag="g")
        nc.tensor.transpose(pst[:], t.v_fm[:, tt * 128 : (tt + 1) * 128], c.ident[:])
        if b == 0:
            nc.scalar.copy(t.v0[:, tt, 0:64], pst[:, 0:64])
            nc.scalar.copy(t.v1[:, tt, 0:64], pst[:, 64:128])
        else:
            nc.vector.tensor_copy(t.v0[:, tt, 0:64], pst[:, 0:64])
            nc.vector.tensor_copy(t.v1[:, tt, 0:64], pst[:, 64:128])


def _phase2_chunk(c, b, qi, fillers=None):
    nc = c.nc
    t = _tiles(c, b)
    qs = slice(qi * QBLK, (qi + 1) * QBLK)
    o0 = c.ps_o.tile([96, QBLK], F32, tag="o")
    o1 = c.ps_o.tile([96, QBLK], F32, tag="o")
    for kj in range(NKJ):
        if fillers and kj in fillers:
            for f in fillers[kj]:
                f()
        ks = slice(kj * 128, (kj + 1) * 128)
        s_ps = c.ps_s.tile([128, 2 * QBLK], F32, tag="s")
        nc.tensor.matmul(
            s_ps[:, 0:QBLK], t.k_ro[0:64, ks], t.q_ro[0:64, qs],
            start=True, stop=True,
        )
        nc.tensor.matmul(
            s_ps[:, QBLK : 2 * QBLK],
            t.k_ro[64:128, ks],
            t.q_ro[64:128, qs],
            start=True,
            stop=True,
            tile_position=(64, 0),
        )
        p = c.pexp.tile([128, 2 * QBLK], F16, tag="p")
        nc.scalar.activation(p[:], s_ps[:], Exp)
        nc.tensor.matmul(
            o0[0:65, :], t.v0[:, kj, 0:65], p[:, 0:QBLK],
            start=(kj == 0), stop=(kj == NKJ - 1),
        )
        nc.tensor.matmul(
            o1[0:65, :], t.v1[:, kj, 0:65], p[:, QBLK : 2 * QBLK],
            start=(kj == 0), stop=(kj == NKJ - 1),
        )
    # Early cross-engine copies release the o-psum banks in ~1.2us (head0 on
    # ACT, head1 on DVE); the recip/broadcast/normalize chain then runs off
    # the PE critical path (denominator staged via a 32-aligned [64:96] copy
    # because custom-DVE ops only read partition-0-based APs). The last chunk
    # runs in two column-halves so the final projection can start on the
    # first half while the second is still normalizing.
    splits = 2 if (b == B - 1 and qi == NQB - 1) else 1
    w = QBLK // splits
    for h in range(splits):
        hs = slice(h * w, (h + 1) * w)
        qh = slice(qi * QBLK + h * w, qi * QBLK + (h + 1) * w)
        stg0 = c.norm.tile([32, w], F32, tag="stg")
        stg1 = c.norm.tile([32, w], F32, tag="stg")
        nc.scalar.copy(t.ao[0:64, qh], o0[0:64, hs])
        nc.scalar.copy(stg0[:], o0[64:96, hs])
        nc.vector.tensor_copy(t.ao[64:128, qh], o1[0:64, hs])
        nc.vector.tensor_copy(stg1[:], o1[64:96, hs])
        rb_full = c.norm.tile([128, w], F32, tag="rbf")
        rbt = c.norm.tile([64, w], F32, tag="rbt")
        for stg, dst in ((stg0, rb_full[0:64, :]), (stg1, rbt[:])):
            r = c.norm.tile([1, w], F32, tag="r")
            nc.vector.reciprocal_approx_fast(r[:], stg[0:1, :])
            nc.gpsimd.partition_broadcast(dst, r[:])
        nc.vector.tensor_copy(rb_full[64:128, :], rbt[:])
        nc.vector.tensor_mul(t.ao[:, qh], t.ao[:, qh], rb_full[:])


def _phase3_part(c, b, blk, e0, e1, last=False):
    nc = c.nc
    t = _tiles(c, b)
    splits = 2 if last else 1
    w = BLK // splits
    for h in range(splits):
        ts = slice(blk * BLK + h * w, blk * BLK + (h + 1) * w)
        for e in range(e0, e1):
            psf = c.ps_g.tile([128, w], F32, tag="g")
            nc.tensor.matmul(
                psf[:],
                c.wp_sb[:, e * 128 : (e + 1) * 128],
                t.ao[:, ts],
                start=True,
                stop=True,
            )
            o_sb = c.outp.tile([128, w], RF16, tag="os")
            # drain the tail on two engines: ACT is exp-free by then
            if last and e % 2 == 1:
                nc.scalar.copy(o_sb[:], psf[:])
            else:
                nc.vector.tensor_copy(o_sb[:], psf[:])
            nc.sync.dma_start(out=c.out_d[b, e * 128 : (e + 1) * 128, ts], in_=o_sb[:])


def _phase3_chunk(c, b, blk):
    _phase3_part(c, b, blk, 0, D // 128, last=(b == B - 1 and blk == NBLK - 1))


_PROGRAM = None


def _program():
    global _PROGRAM
    if _PROGRAM is None:
        _PROGRAM = _build_program()
    return _PROGRAM


def _rope_tables():
    f = np.arange(32, dtype=np.float64)
    inv = ROPE_BASE ** (-2.0 * f / HD)
    t = np.arange(L, dtype=np.float64)
    ang = np.outer(inv, t)  # [32, L]
    cosT = np.cos(ang)
    sinT = np.sin(ang)
    cc = np.tile(cosT, (4, 1)).astype(np.float32)  # [128, L]
    ssw = np.concatenate([sinT, -sinT, sinT, -sinT], axis=0).astype(np.float32)
    return cc, ssw


def _prep_in_maps(x, W_qkv, W_proj):
    xt = np.ascontiguousarray(x.reshape(TOK, D).T).astype(ml_dtypes.bfloat16)
    cc, ssw = _rope_tables()
    scale = HD**-0.5

    evens = np.arange(0, HD, 2)
    odds = np.arange(1, HD, 2)
    in_maps = []
    for c in range(NCORES):
        h0, h1 = HPC * c, HPC * c + 1
        rows_pair = np.concatenate(
            [h0 * HD + evens, h0 * HD + odds, h1 * HD + evens, h1 * HD + odds]
        )
        wq = (W_qkv[rows_pair, :].astype(np.float64) * scale).T  # [D, 128]
        wk = W_qkv[D + rows_pair, :].T  # [D, 128]
        wqk = np.concatenate([wq, wk], axis=1).astype(ml_dtypes.bfloat16)
        rows_v = np.concatenate(
            [2 * D + h0 * HD + np.arange(HD), 2 * D + h1 * HD + np.arange(HD)]
        )
        wv = np.ascontiguousarray(W_qkv[rows_v, :].T).astype(ml_dtypes.bfloat16)  # [D, 128]
        d_rows = np.concatenate([h0 * HD + np.arange(HD), h1 * HD + np.arange(HD)])
        wp = np.ascontiguousarray(W_proj[:, d_rows].T).astype(ml_dtypes.bfloat16)  # [128, D]
        in_maps.append(
            {"xt": xt, "wqk": wqk, "wv": wv, "wp": wp, "cc": cc, "ssw": ssw}
        )
    return in_maps


def run(x, W_qkv, W_proj, trace=False):
    nc = _program()
    in_maps = _prep_in_maps(np.asarray(x), np.asarray(W_qkv), np.asarray(W_proj))
    res = run_bass_kernel_spmd(
        nc, in_maps, core_ids=list(range(NCORES)), trace=trace
    )
    acc = res.results[0]["out"].astype(np.float64)
    for c in range(1, NCORES):
        acc += res.results[c]["out"]
    full = np.transpose(acc, (0, 2, 1)).astype(np.float32)  # [B, L, D]
    return full, res


def kernel(x, W_qkv, W_proj):
    out, _ = run(x, W_qkv, W_proj, trace=False)
    return out


# revision 40
# speedup vs baseline: 1.0271x; 1.0271x over previous
"""Multi-head attention with RoPE on 8 Trainium2 NeuronCores.

Problem: B=4, L=2048, D=1024, H=16 heads of dim 64, fp32, full (non-causal)
softmax attention with concatenated-halves RoPE on q and k.

Sharding: tensor-parallel over heads. Each of the 8 cores owns 2 heads:
 - computes q/k/v projections for its heads only (W_qkv column slice),
 - runs attention for its 2 heads x 4 batches,
 - computes a rank-128 partial of the output projection (W_proj row slice).
The host sums the 8 partial outputs (the only cross-core reduction).

On-core layout choices (v2 — engine-rebalanced):
 - q, k are produced FEATURE-major ([head_dim, tokens]) directly by the QKV
   GEMM (weights pre-transposed/permuted on host), so the QK^T matmul needs
   no transposes. RoPE's even/odd feature split is pre-applied as a row
   permutation of W_q/W_k; RoPE = 2 DVE muls + a 32-partition-block DMA swap
   + a Pool (gpsimd) add, keeping the ACT engine free.
 - scores are computed TRANSPOSED ([k_tokens, q_tokens]); softmax exp is the
   ONLY work on ACT (scale folded into W_q on the host). The denominator
   comes free as a ones-column in the p@v stationary, placed FIRST (psum
   partition 0) so reciprocal_approx_fast can read it straight from PSUM.
 - v is produced feature-major then moved token-major by DMA XBAR transposes
   (no PE transposes, no PSUM traffic). v tiles are padded to 80 cols/kj so
   every transpose lands 16-column-aligned; stationary reads cols 15:80
   (ones at 15, v at 16:80).
 - softmax normalization is fused into the PSUM->SBUF copy of the attention
   output: ao = o_psum * broadcast(1/denom) in one DVE tensor_mul per head.
 - matmul operands are bf16 (PE streams 1 cycle/row); accumulation fp32 in
   PSUM; softmax/rope/normalization math fp32.
 - input/weight/output DMAs are split fine-grained and alternated across the
   two HWDGE queues (sync + scalar) so the opening GEMM starts ~1us in and
   no queue sees head-of-line blocking.
 - emission is software-pipelined: phase1 of batch b+1 and the output
   projection of batch b are emitted between the attention chunks of batch
   b so the Tile scheduler interleaves them into ACT-bound gaps.
"""

import sys

for _p in ("/opt/trn_rl_repo",):
    if _p not in sys.path:
        sys.path.insert(0, _p)

import numpy as np
import concourse.bass as bass
import concourse.mybir as mybir
from concourse import bacc
from concourse.tile import TileContext
from concourse.bass_utils import run_bass_kernel_spmd
from concourse.masks import make_identity

F32 = mybir.dt.float32
import ml_dtypes
F16 = mybir.dt.bfloat16
RF16 = mybir.dt.float16

B, L, D = 4, 2048, 1024
H, HD = 16, 64
NCORES = 8
HPC = H // NCORES  # 2 heads per core
TOK = B * L
BLK = 512  # gemm moving-dim block
QBLK = 512  # attention query block (one PSUM bank of fp32 output)
NBLK = L // BLK  # 4
NQB = L // QBLK  # 4
KT = D // 128  # 8 contraction tiles for the qkv projection
NKJ = L // 128  # 16 key tiles per batch
VW = 65  # v-tile width per kj block: v at 0:64, ones column at 64
ROPE_BASE = 10000.0

Exp = mybir.ActivationFunctionType.Exp


class _Ctx:
    pass


def _build_program():
    nc = bacc.Bacc("TRN2", target_bir_lowering=False, debug=False)

    c = _Ctx()
    c.nc = nc
    c.xt_d = nc.dram_tensor("xt", [D, TOK], F16, kind="ExternalInput")
    c.wqk_d = nc.dram_tensor("wqk", [D, 256], F16, kind="ExternalInput")
    c.wv_d = nc.dram_tensor("wv", [D, 128], F16, kind="ExternalInput")
    c.wp_d = nc.dram_tensor("wp", [128, D], F16, kind="ExternalInput")
    c.cc_d = nc.dram_tensor("cc", [128, L], F32, kind="ExternalInput")
    c.ssw_d = nc.dram_tensor("ssw", [128, L], F32, kind="ExternalInput")
    c.perm_d = nc.dram_tensor("perm", [128, 128], F16, kind="ExternalInput")
    c.out_d = nc.dram_tensor("out", [B, D, L], RF16, kind="ExternalOutput")

    with TileContext(nc) as tc:
        with (
            tc.tile_pool(name="singles", bufs=1) as singles,
            tc.tile_pool(name="xin", bufs=2) as xin,
            tc.tile_pool(name="batch", bufs=2) as batch,
            tc.tile_pool(name="rope", bufs=4) as rope,
            tc.tile_pool(name="pexp", bufs=6) as pexp,
            tc.tile_pool(name="norm", bufs=4) as norm,
            tc.tile_pool(name="outp", bufs=6) as outp,
            tc.tile_pool(name="ps_g", bufs=2, space="PSUM") as ps_g,
            tc.tile_pool(name="ps_s", bufs=2, space="PSUM") as ps_s,
            tc.tile_pool(name="ps_o", bufs=2, space="PSUM") as ps_o,
        ):
            c.xin, c.batch, c.rope = xin, batch, rope
            c.pexp, c.norm, c.outp = pexp, norm, outp
            c.ps_g, c.ps_s, c.ps_o = ps_g, ps_s, ps_o

            # Resident weights / tables. Queue plan:
            #   sync:   wqk (per-kd), wv, cc/ssw (per-blk interleaved), wp
            #   scalar: x batch 0 (first block per-kd)
            # so the opening q-gemm starts as soon as wqk[kd0] + x[kd0] land.
            c.wqk_sb = singles.tile([128, KT, 256], F16, tag="wqk")
            wqk_r = c.wqk_d[:, :].rearrange("(k p) e -> p k e", p=128)
            for kd in range(KT):
                nc.sync.dma_start(out=c.wqk_sb[:, kd, :], in_=wqk_r[:, kd, :])
            c.bt = {}
            _issue_x(c, 0)
            c.wv_sb = singles.tile([128, KT, 128], F16, tag="wv")
            nc.sync.dma_start(
                out=c.wv_sb[:], in_=c.wv_d[:, :].rearrange("(k p) e -> p k e", p=128)
            )
            c.cc_sb = singles.tile([128, L], F32, tag="cc")
            c.ssw_sb = singles.tile([128, L], F32, tag="ssw")

            def _issue_tables(blk):
                ts = slice(blk * BLK, (blk + 1) * BLK)
                nc.sync.dma_start(out=c.cc_sb[:, ts], in_=c.cc_d[:, ts])
                nc.sync.dma_start(out=c.ssw_sb[:, ts], in_=c.ssw_d[:, ts])
                if blk > 0:
                    t0 = c.bt[0]
                    x_r = c.xt_d[:, 0:L].rearrange("(k p) t -> p k t", p=128)
                    nc.sync.dma_start(out=t0.x_t[:, :, ts], in_=x_r[:, :, ts])

            c.wp_sb = singles.tile([128, D], F16, tag="wp")
            c.perm_sb = singles.tile([128, 128], F16, tag="perm")
            nc.sync.dma_start(out=c.perm_sb[:], in_=c.perm_d[:, :])
            c.ident = singles.tile([128, 128], F16, tag="ident")
            make_identity(nc, c.ident[:])

            # Software-pipelined emission. Batch 0's phase1 runs standalone
            # (q first on block 0 so attention can start; k before q on later
            # blocks since scores consume every k block in qi order); the v
            # transposes ride right behind each v_fm block. Rope tables land
            # just-in-time so the sync queue reaches the swap DMAs without
            # backlog.
            _vinit(c, 0)
            for blk in range(NBLK):
                _issue_tables(blk)
                order = "qkv" if blk == 0 else "kvq"
                for g in order:
                    _gemm_group(c, 0, blk, g)
                    if g == "v":
                        _vtrans_part(c, 0, blk)
            nc.sync.dma_start(out=c.wp_sb[:], in_=c.wp_d[:, :])

            # Main loop: each phase2 kj-stream carries interleaved "filler"
            # PE work (next batch's gemm groups, v transposes, and the
            # previous chunk's projection) so the Tensor engine has
            # exp-independent matmuls to run while ACT catches up.
            for b in range(B):
                for qi in range(NQB):
                    fillers = {}
                    if b + 1 < B:
                        if qi == 0:
                            _vinit(c, b + 1)
                            _issue_x(c, b + 1)

                        def mk(g, bb=b + 1, blkx=qi):
                            def f():
                                _gemm_group(c, bb, blkx, g)
                                if g == "v":
                                    _vtrans_part(c, bb, blkx)
                            return f

                        fillers[3] = [mk("v")]
                        fillers[6] = [mk("q")]
                        fillers[9] = [mk("k")]
                    if qi >= 1:
                        pb, pblk = b, qi - 1
                    elif b > 0:
                        pb, pblk = b - 1, NQB - 1
                    else:
                        pb = None
                    if pb is not None:
                        fillers.setdefault(12, []).append(
                            lambda bb=pb, blkx=pblk: _phase3_part(c, bb, blkx, 0, 4)
                        )
                        fillers.setdefault(15, []).append(
                            lambda bb=pb, blkx=pblk: _phase3_part(c, bb, blkx, 4, 8)
                        )
                    _phase2_chunk(c, b, qi, fillers)
            _phase3_chunk(c, B - 1, NQB - 1)

    nc.compile()
    return nc


def _tiles(c, b):
    if b not in c.bt:
        t = _Ctx()
        t.q_ro = c.batch.tile([128, L], F16, tag="qro")
        t.k_ro = c.batch.tile([128, L], F16, tag="kro")
        t.v_fm = c.batch.tile([128, L], F16, tag="vfm")
        t.v0 = c.batch.tile([128, NKJ, VW], F16, tag="v0")
        t.v1 = c.batch.tile([128, NKJ, VW], F16, tag="v1")
        t.ao = c.batch.tile([128, L], F16, tag="ao")
        t.x_t = None
        c.bt[b] = t
    return c.bt[b]


def _issue_x(c, b):
    # chunked x load on the scalar queue (sync carries weights/tables/etc).
    # Batch 0's first block is issued per-kd so the opening gemm only waits
    # for ~128KB; later chunks keep full prefetch lead time.
    nc = c.nc
    t = _tiles(c, b)
    t.x_t = c.xin.tile([128, KT, L], F16, tag="x")
    x_r = c.xt_d[:, b * L : (b + 1) * L].rearrange("(k p) t -> p k t", p=128)
    if b == 0:
        # fine pieces for the opening gemm; blocks 1-3 are emitted by the
        # phase1 loop (on sync, interleaved with the rope tables) so the
        # scalar queue reaches batch 0's swap DMAs early
        for kd in range(KT):
            nc.scalar.dma_start(
                out=t.x_t[:, kd, 0:BLK], in_=x_r[:, kd, 0:BLK]
            )
        return
    for ck in range(NBLK):
        nc.scalar.dma_start(
            out=t.x_t[:, :, ck * BLK : (ck + 1) * BLK],
            in_=x_r[:, :, ck * BLK : (ck + 1) * BLK],
        )


def _gemm_group(c, b, blk, g):
    nc = c.nc
    t = _tiles(c, b)
    ts = slice(blk * BLK, (blk + 1) * BLK)
    if t.x_t is None:
        _issue_x(c, b)
    if g in ("q", "k"):
        wcol, dst = (0, t.q_ro) if g == "q" else (128, t.k_ro)
        ps = c.ps_g.tile([128, BLK], F32, tag="g")
        for kd in range(KT):
            nc.tensor.matmul(
                ps[:],
                c.wqk_sb[:, kd, wcol : wcol + 128],
                t.x_t[:, kd, ts],
                start=(kd == 0),
                stop=(kd == KT - 1),
            )
        # rope: dst = ps*cc + blockswap(ps*ssw); muls on DVE, the 32-row
        # block swap as a one-hot permutation matmul on the PE (512 cycles —
        # far cheaper than SBUF->SBUF DMAs + queue/semaphore latency), add
        # on DVE reading the permuted PSUM directly.
        tmp_c = c.rope.tile([128, BLK], F32, tag="tc")
        nc.vector.tensor_mul(tmp_c[:], ps[:], c.cc_sb[:, ts])
        tmp_s = c.rope.tile([128, BLK], F16, tag="tsn")
        nc.vector.tensor_mul(tmp_s[:], ps[:], c.ssw_sb[:, ts])
        psw = c.ps_g.tile([128, BLK], F32, tag="g")
        nc.tensor.matmul(psw[:], c.perm_sb[:], tmp_s[:], start=True, stop=True)
        nc.vector.tensor_add(dst[:, ts], tmp_c[:], psw[:])
    else:
        psv = c.ps_g.tile([128, BLK], F32, tag="g")
        for kd in range(KT):
            nc.tensor.matmul(
                psv[:],
                c.wv_sb[:, kd, :],
                t.x_t[:, kd, ts],
                start=(kd == 0),
                stop=(kd == KT - 1),
            )
        # during batch 0's phase1 ACT is idle (no exp yet) — use it so the
        # DVE backlog doesn't delay k_ro/v readiness for the first phase2
        if b == 0:
            nc.scalar.copy(t.v_fm[:, ts], psv[:])
        else:
            nc.vector.tensor_copy(t.v_fm[:, ts], psv[:])


def _vinit(c, b):
    nc = c.nc
    t = _tiles(c, b)
    nc.gpsimd.memset(t.v0[:, :, 64], 1.0)
    nc.gpsimd.memset(t.v1[:, :, 64], 1.0)


def _vtrans_part(c, b, blk):
    nc = c.nc
    t = _tiles(c, b)
    for tt in range(blk * 4, blk * 4 + 4):
        pst = c.ps_g.tile([128, 128], F16, tag="g")
        nc.tensor.transpose(pst[:], t.v_fm[:, tt * 128 : (tt + 1) * 128], c.ident[:])
        if b == 0:
            nc.scalar.copy(t.v0[:, tt, 0:64], pst[:, 0:64])
            nc.scalar.copy(t.v1[:, tt, 0:64], pst[:, 64:128])
        else:
            nc.vector.tensor_copy(t.v0[:, tt, 0:64], pst[:, 0:64])
            nc.vector.tensor_copy(t.v1[:, tt, 0:64], pst[:, 64:128])


def _phase2_chunk(c, b, qi, fillers=None):
    nc = c.nc
    t = _tiles(c, b)
    qs = slice(qi * QBLK, (qi + 1) * QBLK)
    o0 = c.ps_o.tile([96, QBLK], F32, tag="o")
    o1 = c.ps_o.tile([96, QBLK], F32, tag="o")
    for kj in range(NKJ):
        if fillers and kj in fillers:
            for f in fillers[kj]:
                f()
        ks = slice(kj * 128, (kj + 1) * 128)
        s_ps = c.ps_s.tile([128, 2 * QBLK], F32, tag="s")
        nc.tensor.matmul(
            s_ps[:, 0:QBLK], t.k_ro[0:64, ks], t.q_ro[0:64, qs],
            start=True, stop=True,
        )
        nc.tensor.matmul(
            s_ps[:, QBLK : 2 * QBLK],
            t.k_ro[64:128, ks],
            t.q_ro[64:128, qs],
            start=True,
            stop=True,
            tile_position=(64, 0),
        )
        p = c.pexp.tile([128, 2 * QBLK], F16, tag="p")
        nc.scalar.activation(p[:], s_ps[:], Exp)
        nc.tensor.matmul(
            o0[0:65, :], t.v0[:, kj, 0:65], p[:, 0:QBLK],
            start=(kj == 0), stop=(kj == NKJ - 1),
        )
        nc.tensor.matmul(
            o1[0:65, :], t.v1[:, kj, 0:65], p[:, QBLK : 2 * QBLK],
            start=(kj == 0), stop=(kj == NKJ - 1),
        )
    # Early cross-engine copies release the o-psum banks in ~1.2us (head0 on
    # ACT, head1 on DVE); the recip/broadcast/normalize chain then runs off
    # the PE critical path (denominator staged via a 32-aligned [64:96] copy
    # because custom-DVE ops only read partition-0-based APs). The last chunk
    # runs in two column-halves so the final projection can start on the
    # first half while the second is still normalizing.
    splits = 2 if (b == B - 1 and qi == NQB - 1) else 1
    w = QBLK // splits
    for h in range(splits):
        hs = slice(h * w, (h + 1) * w)
        qh = slice(qi * QBLK + h * w, qi * QBLK + (h + 1) * w)
        stg0 = c.norm.tile([32, w], F32, tag="stg")
        stg1 = c.norm.tile([32, w], F32, tag="stg")
        nc.scalar.copy(t.ao[0:64, qh], o0[0:64, hs])
        nc.scalar.copy(stg0[:], o0[64:96, hs])
        nc.vector.tensor_copy(t.ao[64:128, qh], o1[0:64, hs])
        nc.vector.tensor_copy(stg1[:], o1[64:96, hs])
        rb_full = c.norm.tile([128, w], F32, tag="rbf")
        rbt = c.norm.tile([64, w], F32, tag="rbt")
        for stg, dst in ((stg0, rb_full[0:64, :]), (stg1, rbt[:])):
            r = c.norm.tile([1, w], F32, tag="r")
            nc.vector.reciprocal_approx_fast(r[:], stg[0:1, :])
            nc.gpsimd.partition_broadcast(dst, r[:])
        nc.vector.tensor_copy(rb_full[64:128, :], rbt[:])
        nc.vector.tensor_mul(t.ao[:, qh], t.ao[:, qh], rb_full[:])


def _phase3_part(c, b, blk, e0, e1, last=False):
    nc = c.nc
    t = _tiles(c, b)
    splits = 2 if last else 1
    w = BLK // splits
    for h in range(splits):
        ts = slice(blk * BLK + h * w, blk * BLK + (h + 1) * w)
        for e in range(e0, e1):
            psf = c.ps_g.tile([128, w], F32, tag="g")
            nc.tensor.matmul(
                psf[:],
                c.wp_sb[:, e * 128 : (e + 1) * 128],
                t.ao[:, ts],
                start=True,
                stop=True,
            )
            o_sb = c.outp.tile([128, w], RF16, tag="os")
            # drain the tail on two engines: ACT is exp-free by then
            if last and e % 2 == 1:
                nc.scalar.copy(o_sb[:], psf[:])
            else:
                nc.vector.tensor_copy(o_sb[:], psf[:])
            nc.sync.dma_start(out=c.out_d[b, e * 128 : (e + 1) * 128, ts], in_=o_sb[:])


def _phase3_chunk(c, b, blk):
    _phase3_part(c, b, blk, 0, D // 128, last=(b == B - 1 and blk == NBLK - 1))


_PROGRAM = None


def _program():
    global _PROGRAM
    if _PROGRAM is None:
        _PROGRAM = _build_program()
    return _PROGRAM


def _rope_tables():
    f = np.arange(32, dtype=np.float64)
    inv = ROPE_BASE ** (-2.0 * f / HD)
    t = np.arange(L, dtype=np.float64)
    ang = np.outer(inv, t)  # [32, L]
    cosT = np.cos(ang)
    sinT = np.sin(ang)
    cc = np.tile(cosT, (4, 1)).astype(np.float32)  # [128, L]
    ssw = np.concatenate([sinT, -sinT, sinT, -sinT], axis=0).astype(np.float32)
    return cc, ssw


def _prep_in_maps(x, W_qkv, W_proj):
    xt = np.ascontiguousarray(x.reshape(TOK, D).T).astype(ml_dtypes.bfloat16)
    cc, ssw = _rope_tables()
    scale = HD**-0.5

    # one-hot matrix for the rope 32-partition block swap: out = perm.T @ in
    # gives out[p] = in[sigma(p)], sigma = (0<->32, 64<->96) in 32-blocks
    sigma = np.arange(128)
    sigma = sigma + np.where((sigma // 32) % 2 == 0, 32, -32)
    perm = np.zeros((128, 128), dtype=ml_dtypes.bfloat16)
    perm[sigma, np.arange(128)] = 1

    evens = np.arange(0, HD, 2)
    odds = np.arange(1, HD, 2)
    in_maps = []
    for c in range(NCORES):
        h0, h1 = HPC * c, HPC * c + 1
        rows_pair = np.concatenate(
            [h0 * HD + evens, h0 * HD + odds, h1 * HD + evens, h1 * HD + odds]
        )
        wq = (W_qkv[rows_pair, :].astype(np.float64) * scale).T  # [D, 128]
        wk = W_qkv[D + rows_pair, :].T  # [D, 128]
        wqk = np.concatenate([wq, wk], axis=1).astype(ml_dtypes.bfloat16)
        rows_v = np.concatenate(
            [2 * D + h0 * HD + np.arange(HD), 2 * D + h1 * HD + np.arange(HD)]
        )
        wv = np.ascontiguousarray(W_qkv[rows_v, :].T).astype(ml_dtypes.bfloat16)  # [D, 128]
        d_rows = np.concatenate([h0 * HD + np.arange(HD), h1 * HD + np.arange(HD)])
        wp = np.ascontiguousarray(W_proj[:, d_rows].T).astype(ml_dtypes.bfloat16)  # [128, D]
        in_maps.append(
            {"xt": xt, "wqk": wqk, "wv": wv, "wp": wp, "cc": cc, "ssw": ssw,
             "perm": perm}
        )
    return in_maps


def run(x, W_qkv, W_proj, trace=False):
    nc = _program()
    in_maps = _prep_in_maps(np.asarray(x), np.asarray(W_qkv), np.asarray(W_proj))
    res = run_bass_kernel_spmd(
        nc, in_maps, core_ids=list(range(NCORES)), trace=trace
    )
    acc = res.results[0]["out"].astype(np.float64)
    for c in range(1, NCORES):
        acc += res.results[c]["out"]
    full = np.transpose(acc, (0, 2, 1)).astype(np.float32)  # [B, L, D]
    return full, res


def kernel(x, W_qkv, W_proj):
    out, _ = run(x, W_qkv, W_proj, trace=False)
    return out
